# revision 9
# baseline (speedup 1.0000x reference)
"""Trainium2 Bass kernel for nn_DEC_GNN_Encoder (GATConv + diag-select + FC).

Exact-math restructuring of the reference:
  * The final output only reads the GAT result at the 576 "diagonal" nodes
    (ids = i*n^2 + j*(n+1)), so only edges with dst in ids (~9.8k of 470k)
    participate in the segment softmax / aggregation.  This is an algebraic
    identity (select-before-FC == select-after), not an approximation.
  * a_src/a_dst/a_edge come from host-folded weight vectors
    (x @ W @ att == x @ (W att)); h itself is computed per edge.
  * The softmax max-subtraction is dropped: logits are O(+-6) here, and
    alpha = exp(l)/sum(exp(l)) is shift-invariant, so this is exact.
  * The only computation touching all edges is edge_attr.mean(0) (the PyG
    self-loop attr fill).  Each core reduces 1/8 of edge_attr on device and
    the 16-float partials are combined with an on-device AllReduce.

Sharding: the 576 output nodes (and their incident edges) are partitioned
across the 8 cores, 72 nodes each; x rows for edge endpoints are
host-gathered into per-core inputs (halo exchange at input-staging time);
the small weight matrices are replicated.

dtypes: fp32 data pre-rounded to 10 mantissa bits and declared float32r so
the big matmuls stream at 1 col/cycle (fp32 runs a 4x-slower 2-pass mode);
the aggregation matmuls and edge_attr-mean reduce run in bf16 (exact for
the 0/1 onehot operands; ~4e-3 elsewhere, averaged down by the softmax).

Device pipeline per core (single NEFF):
  M) edge_attr mean partials (DVE reduce over a host-transposed bf16
     layout) -> AllReduce -> self-loop a_edge row
  B) per 128-edge block: h = x@W (PE, f32r) -> bf16 [h|1]x4 layout
  Z) logits in [4, edges] orientation: one PSUM accumulation per 512-edge
     chunk with tiny stationaries (usrc/v16/adst/aerow), lrelu 0.2 + exp,
     PE-transpose of exp back to edge-major
  C) segment-sum + softmax denominator fused: psum[n, 0:129] +=
     (expl*onehot)_h^T @ [h|1]  (bf16)
  D) out = psum[:,0:128]/(S+1e-16) + bias_gat; lrelu 0.01 (DVE)
  E) FC: PE-transpose h2, psum_fc = sum_k h2^T_k @ fc_W_k + ones^T @ fc_b,
     lrelu 0.01, DMA out.

Self-loop edges sit at the END of the per-core edge list so everything
that depends on the AllReduce (their a_edge -> logits -> aggregation
closure) is the tail of the pipeline; blocks of real edges overlap the
collective latency.
"""

import numpy as np

N_CORES = 8
HEADS = 4
C = 128
HC = HEADS * C
NEG_ATT = 0.2
NEG = 0.01

_CACHE = {}


def _round10(x):
    """Round fp32 to 10 mantissa bits (float32r-compatible pre-rounding)."""
    x = np.ascontiguousarray(x, np.float32)
    u = x.view(np.uint32)
    return ((u + 0x0FFF + ((u >> 13) & 1)) & 0xFFFFE000).view(np.float32)


def _build_program(n_loc, e_cap, f_pp, nch, inv_e):
    import concourse.mybir as mybir
    import concourse.tile as tile
    from concourse import bacc
    from concourse.masks import make_identity

    F32 = mybir.dt.float32
    F32R = mybir.dt.float32r
    BF16 = mybir.dt.bfloat16
    AL = mybir.AluOpType
    ACT = mybir.ActivationFunctionType
    nblk = e_cap // 128
    cw_m = f_pp // nch  # free width of one mean chunk

    # logits chunks cover only the real-edge columns [0, sl0); the
    # self-loop block (last 128 columns) is handled as a diagonal update
    sl0 = e_cap - 128
    chunks = []
    off = 0
    while off < sl0:
        w = min(512, sl0 - off)
        chunks.append((off, w))
        off += w

    nc = bacc.Bacc("TRN2", target_bir_lowering=False, debug=False,
                   num_devices=N_CORES)

    def din(name, shape, dt):
        return nc.dram_tensor(name, list(shape), dt, kind="ExternalInput").ap()

    d_xt = din("xt", (256, e_cap), F32R)
    d_eat = din("eat", (16, e_cap), F32R)
    d_eqt = din("eqt", (n_loc, e_cap - 128), F32R)
    d_eqb = din("eqb", (128, (nblk - 1) * n_loc), F32)
    d_mea = din("mea", (128, f_pp), BF16)
    d_w = din("w", (256, HC), F32R)
    d_usrc = din("usrc", (256, HEADS), F32R)
    d_udst = din("udst", (256, HEADS), F32R)
    d_usd = din("usd", (256, HEADS), F32R)
    d_v16 = din("v16", (16, HEADS), F32R)
    d_v16f = din("v16f", (16, HEADS), F32)
    d_fcw = din("fcw", (HC, 256), F32R)
    d_fcb = din("fcb", (1, 256), F32R)
    d_biasb = din("biasb", (n_loc, HC), F32)
    d_onesr = din("onesr", (1, 128), F32R)
    d_onesc = din("onesc", (128, 1), F32)
    d_out = nc.dram_tensor("out", [n_loc, 256], F32, kind="ExternalOutput").ap()

    with tile.TileContext(nc) as tc:
        with (
            tc.tile_pool(name="cst", bufs=1) as cst,
            tc.tile_pool(name="pb", bufs=1) as pb,
            tc.tile_pool(name="tmp", bufs=2) as tmp,
            tc.tile_pool(name="psh", bufs=2, space="PSUM") as psh,
            tc.tile_pool(name="psl", bufs=1, space="PSUM") as psl,
            tc.tile_pool(name="pso", bufs=1, space="PSUM") as pso,
            tc.tile_pool(name="dr", bufs=1, space="DRAM") as dr,
        ):
            def load(dram, shape, tag, dt):
                t = cst.tile(list(shape), dt, tag=tag, name=tag)
                nc.sync.dma_start(t[:], dram[:])
                return t

            # mean-input DMAs first, on the otherwise-idle SWDGE queue, so
            # the AllReduce chain launches as early as possible
            tcw = (f_pp // nch) // 16
            maccs = []
            for ci in range(nch):
                cw0 = f_pp // nch
                mc = tmp.tile([128, cw0], BF16, tag="meac", name="meac")
                nc.gpsimd.dma_start(mc[:], d_mea[:, ci * cw0:(ci + 1) * cw0])
                acc = pb.tile([128, 16], F32, tag=f"macc{ci}",
                              name=f"macc{ci}")
                nc.vector.reduce_sum(
                    out=acc[:],
                    in_=mc[:].rearrange("p (j t) -> p j t", t=tcw),
                    axis=mybir.AxisListType.X)
                maccs.append(acc)

            t_w0 = load(d_w[0:128, :], (128, HC), "w0", F32R)
            t_w1 = load(d_w[128:256, :], (128, HC), "w1", F32R)
            t_xt0 = load(d_xt[0:128, :], (128, e_cap), "xt0", F32R)
            t_xt1 = load(d_xt[128:256, :], (128, e_cap), "xt1", F32R)
            t_us0 = load(d_usrc[0:128, :], (128, HEADS), "us0", F32R)
            t_us1 = load(d_usrc[128:256, :], (128, HEADS), "us1", F32R)
            t_ud0 = load(d_udst[0:128, :], (128, HEADS), "ud0", F32R)
            t_ud1 = load(d_udst[128:256, :], (128, HEADS), "ud1", F32R)
            t_usd0 = load(d_usd[0:128, :], (128, HEADS), "usd0", F32R)
            t_usd1 = load(d_usd[128:256, :], (128, HEADS), "usd1", F32R)
            t_v16 = load(d_v16, (16, HEADS), "v16", F32R)
            t_v16f = load(d_v16f, (16, HEADS), "v16f", F32)
            t_eat = load(d_eat, (16, e_cap), "eat", F32R)
            t_eqt = load(d_eqt, (n_loc, e_cap - 128), "eqt", F32R)
            t_eqb = load(d_eqb, (128, (nblk - 1) * n_loc), "eqb", F32)
            t_fcw = [load(d_fcw[k * 128:(k + 1) * 128, :], (128, 256),
                          f"fcw{k}", F32R) for k in range(4)]
            t_fcb = load(d_fcb, (1, 256), "fcb", F32R)
            t_biasb = load(d_biasb, (n_loc, HC), "biasb", F32)
            t_onesr = load(d_onesr, (1, 128), "onesr", F32R)
            t_onesc = load(d_onesc, (128, 1), "onesc", F32)
            t_id = cst.tile([128, 128], F32, tag="ident", name="ident")
            make_identity(nc, t_id[:])

            # ---- stage B: per-block h -> bf16 [h|1]x4 layout ----
            hsbs = {}
            for b in range(nblk):
                xs = slice(b * 128, (b + 1) * 128)
                p_h = psh.tile([128, HC], F32, tag="h", name="p_h")
                nc.tensor.matmul(p_h[:], lhsT=t_xt0[:, xs], rhs=t_w0[:],
                                 start=True, stop=False)
                nc.tensor.matmul(p_h[:], lhsT=t_xt1[:, xs], rhs=t_w1[:],
                                 start=False, stop=True)
                hsb = pb.tile([128, HEADS * 129], BF16, tag=f"hsb{b}",
                              name=f"hsb{b}")
                hview = hsb[:].rearrange("p (a b) -> p a b", b=129)
                nc.scalar.copy(hview[:, :, 0:128],
                               p_h[:].rearrange("p (a b) -> p a b", b=128))
                nc.vector.memset(hview[:, :, 128:129], 1.0)
                hsbs[b] = hsb

            # ---- stage A: a_dst table from the self-loop columns ----
            # (first user of the "l" psum slot: nothing collective-bound
            # may precede it on this slot's rotation chain)
            p_t = psl.tile([n_loc, HEADS], F32, tag="l", name="p_t")
            nc.tensor.matmul(p_t[:], lhsT=t_xt0[:, sl0:sl0 + n_loc],
                             rhs=t_ud0[:], start=True, stop=False)
            nc.tensor.matmul(p_t[:], lhsT=t_xt1[:, sl0:sl0 + n_loc],
                             rhs=t_ud1[:], start=False, stop=True)
            adst = pb.tile([n_loc, HEADS], F32R, tag="adst", name="adst")
            nc.scalar.copy(adst[:], p_t[:])

            # pre-collective part of the self-loop logits, [4, n_loc]
            p_zl = psl.tile([HEADS, n_loc], F32, tag="l", name="p_zl")
            nc.tensor.matmul(p_zl[:], lhsT=t_usd0[:],
                             rhs=t_xt0[:, sl0:sl0 + n_loc],
                             start=True, stop=False)
            nc.tensor.matmul(p_zl[:], lhsT=t_usd1[:],
                             rhs=t_xt1[:, sl0:sl0 + n_loc],
                             start=False, stop=True)
            zlT_pre = pb.tile([HEADS, n_loc], F32, tag="zlT_pre",
                              name="zlT_pre")
            nc.scalar.copy(zlT_pre[:], p_zl[:])

            # ---- stage M tail: partials -> AllReduce (own psum tag) ----
            p_m = psl.tile([16, 1], F32, tag="ae", name="p_m")
            for ci in range(nch):
                nc.tensor.matmul(p_m[:], lhsT=maccs[ci][:], rhs=t_onesc[:],
                                 start=(ci == 0), stop=(ci == nch - 1))
            pm_sb = pb.tile([16, 1], F32, tag="pm_sb", name="pm_sb")
            nc.scalar.activation(pm_sb[:], p_m[:], ACT.Copy,
                                 scale=float(inv_e))
            cb_in = dr.tile([16, 1], F32, name="cb_in")
            cb_out = dr.tile([16, 1], F32, name="cb_out")
            nc.gpsimd.dma_start(cb_in[:], pm_sb[:])
            nc.gpsimd.collective_compute(
                "AllReduce", AL.add,
                replica_groups=[list(range(N_CORES))],
                ins=[cb_in.opt()], outs=[cb_out.opt()],
            )

            # ---- stages Z+C interleaved by chunk ----
            p_outs = [pso.tile([n_loc, 129], F32, tag=f"out{h}",
                               name=f"p_out{h}") for h in range(HEADS)]

            for ci, (coff, cw) in enumerate(chunks):
                cs = slice(coff, coff + cw)
                p_z = psl.tile([HEADS, cw], F32, tag="l", name="p_z")
                nc.tensor.matmul(p_z[:], lhsT=t_us0[:], rhs=t_xt0[:, cs],
                                 start=True, stop=False,
                                 skip_group_check=True)
                nc.tensor.matmul(p_z[:], lhsT=t_us1[:], rhs=t_xt1[:, cs],
                                 start=False, stop=False,
                                 skip_group_check=True)
                nc.tensor.matmul(p_z[:], lhsT=t_v16[:], rhs=t_eat[:, cs],
                                 start=False, stop=False,
                                 skip_group_check=True)
                nc.tensor.matmul(p_z[:], lhsT=adst[:], rhs=t_eqt[:, cs],
                                 start=False, stop=True,
                                 skip_group_check=True)
                zsb = tmp.tile([HEADS, cw], F32, tag="zsb", name="zsb")
                nc.scalar.copy(zsb[:], p_z[:])
                zlrelu = tmp.tile([HEADS, cw], F32, tag="zlrelu",
                                  name="zlrelu")
                nc.vector.scalar_tensor_tensor(
                    out=zlrelu[:], in0=zsb[:], scalar=NEG_ATT, in1=zsb[:],
                    op0=AL.mult, op1=AL.max)
                explT = tmp.tile([HEADS, cw], F32, tag="explT",
                                 name="explT")
                nc.scalar.activation(explT[:], zlrelu[:], ACT.Exp)

                for b in range(coff // 128, (coff + cw) // 128):
                    p_te = psh.tile([128, HEADS], F32, tag="h", name="p_te")
                    nc.tensor.transpose(
                        p_te[:],
                        in_=explT[:, b * 128 - coff:(b + 1) * 128 - coff],
                        identity=t_id[0:HEADS, 0:HEADS])
                    eqh4 = tmp.tile([128, HEADS * n_loc], BF16,
                                    tag="eqh4", name="eqh4", bufs=3)
                    teb = p_te[:].rearrange("p (a b) -> p a b", b=1) \
                        .to_broadcast([128, HEADS, n_loc])
                    eqv = t_eqb[:, b * n_loc:(b + 1) * n_loc] \
                        .rearrange("p (a b) -> p a b", a=1) \
                        .to_broadcast([128, HEADS, n_loc])
                    nc.vector.scalar_tensor_tensor(
                        out=eqh4[:], in0=teb, scalar=1.0, in1=eqv,
                        op0=AL.mult, op1=AL.mult)
                    for h in range(HEADS):
                        nc.tensor.matmul(
                            p_outs[h][:],
                            lhsT=eqh4[:, h * n_loc:(h + 1) * n_loc],
                            rhs=hsbs[b][:, h * 129:(h + 1) * 129],
                            start=(b == 0), stop=(b == nblk - 2),
                            skip_group_check=True)

            # ---- post-collective: finish self-loop logits ----
            gsum = pb.tile([16, 1], F32, tag="gsum", name="gsum")
            nc.gpsimd.dma_start(gsum[:], cb_out[:])
            p_ae = psl.tile([HEADS, 1], F32, tag="ae", name="p_ae")
            nc.tensor.matmul(p_ae[:], lhsT=t_v16f[:], rhs=gsum[:],
                             start=True, stop=True)
            zf = tmp.tile([HEADS, n_loc], F32, tag="zf", name="zf")
            nc.vector.tensor_scalar_add(out=zf[:], in0=zlT_pre[:],
                                        scalar1=p_ae[:, 0:1])
            zfl = tmp.tile([HEADS, n_loc], F32, tag="zfl", name="zfl")
            nc.vector.scalar_tensor_tensor(
                out=zfl[:], in0=zf[:], scalar=NEG_ATT, in1=zf[:],
                op0=AL.mult, op1=AL.max)
            explT_l = tmp.tile([HEADS, n_loc], F32, tag="explT_l",
                               name="explT_l")
            nc.scalar.activation(explT_l[:], zfl[:], ACT.Exp)
            p_el = psl.tile([n_loc, HEADS], F32, tag="l", name="p_el")
            nc.tensor.transpose(p_el[:], in_=explT_l[:],
                                identity=t_id[0:HEADS, 0:HEADS])
            expl_l = tmp.tile([n_loc, HEADS], F32, tag="expl_l",
                              name="expl_l")
            nc.scalar.copy(expl_l[:], p_el[:])

            # ---- stage D: diagonal self-loop update + normalize ----
            st = tmp.tile([n_loc, HEADS], F32, tag="st", name="st")
            for h in range(HEADS):
                nc.vector.tensor_scalar_add(
                    out=st[:, h:h + 1], in0=p_outs[h][:, 128:129],
                    scalar1=1e-16)
            sf = tmp.tile([n_loc, HEADS], F32, tag="sf", name="sf")
            nc.vector.tensor_add(sf[:], st[:], expl_l[:])
            rec = tmp.tile([n_loc, HEADS], F32, tag="rec", name="rec")
            nc.vector.reciprocal(rec[:], sf[:])
            hl = nblk - 1  # self-loop block
            g = tmp.tile([n_loc, HC], F32, tag="g", name="g")
            gp = tmp.tile([n_loc, HC], F32, tag="gp", name="gp")
            for h in range(HEADS):
                nc.vector.scalar_tensor_tensor(
                    out=gp[:, h * 128:(h + 1) * 128],
                    in0=hsbs[hl][0:n_loc, h * 129:h * 129 + 128],
                    scalar=expl_l[:, h:h + 1],
                    in1=p_outs[h][:, 0:128],
                    op0=AL.mult, op1=AL.add)
                nc.vector.scalar_tensor_tensor(
                    out=g[:, h * 128:(h + 1) * 128],
                    in0=gp[:, h * 128:(h + 1) * 128],
                    scalar=rec[:, h:h + 1],
                    in1=t_biasb[:, h * 128:(h + 1) * 128],
                    op0=AL.mult, op1=AL.add)
            h2 = tmp.tile([n_loc, HC], F32, tag="h2", name="h2")
            for h in range(HEADS):
                nc.vector.scalar_tensor_tensor(
                    out=h2[:, h * 128:(h + 1) * 128],
                    in0=g[:, h * 128:(h + 1) * 128], scalar=NEG,
                    in1=g[:, h * 128:(h + 1) * 128],
                    op0=AL.mult, op1=AL.max)

            # ---- stage E: FC + lrelu(0.01) ----
            h2t = []
            for k in range(4):
                p_tr = psh.tile([128, n_loc], F32, tag="h", name="p_tr")
                nc.tensor.transpose(p_tr[:],
                                    in_=h2[:, k * 128:(k + 1) * 128],
                                    identity=t_id[0:n_loc, 0:n_loc])
                tk = tmp.tile([128, n_loc], F32R, tag=f"h2t{k}",
                              name=f"h2t{k}")
                nc.scalar.copy(tk[:], p_tr[:])
                h2t.append(tk)
            p_fc = psh.tile([n_loc, 256], F32, tag="h", name="p_fc")
            for k in range(4):
                nc.tensor.matmul(p_fc[:], lhsT=h2t[k][:], rhs=t_fcw[k][:],
                                 start=(k == 0), stop=False,
                                 skip_group_check=True)
            nc.tensor.matmul(p_fc[:], lhsT=t_onesr[:, 0:n_loc], rhs=t_fcb[:],
                             start=False, stop=True, skip_group_check=True)
            ofp = tmp.tile([n_loc, 256], F32, tag="ofp", name="ofp")
            nc.scalar.copy(ofp[:], p_fc[:])
            of = tmp.tile([n_loc, 256], F32, tag="of", name="of")
            nc.vector.scalar_tensor_tensor(
                out=of[:], in0=ofp[:], scalar=NEG, in1=ofp[:],
                op0=AL.mult, op1=AL.max)
            nc.sync.dma_start(d_out[:], of[:])

    nc.compile()
    return nc


def _host_prep(x, edge_index, edge_attr, num_groups, agents_per_group,
               W, att_src, att_dst, W_edge, att_edge, bias_gat, fc_W, fc_b):
    import ml_dtypes

    x = np.ascontiguousarray(np.asarray(x, np.float32))
    edge_index = np.asarray(edge_index)
    edge_attr = np.ascontiguousarray(np.asarray(edge_attr, np.float32))
    W = np.asarray(W, np.float32)
    att_src = np.asarray(att_src, np.float32)
    att_dst = np.asarray(att_dst, np.float32)
    W_edge = np.asarray(W_edge, np.float32)
    att_edge = np.asarray(att_edge, np.float32)
    bias_gat = np.asarray(bias_gat, np.float32)
    fc_W = np.asarray(fc_W, np.float32)
    fc_b = np.asarray(fc_b, np.float32)

    N, f_in = x.shape
    E = edge_index.shape[1]
    ng = int(np.asarray(num_groups))
    na = int(np.asarray(agents_per_group))
    assert ng * na * na == N
    ids = (np.arange(ng, dtype=np.int64)[:, None] * (na * na)
           + np.arange(na, dtype=np.int64)[None, :] * (na + 1)).reshape(-1)
    n_out = ids.size
    assert n_out % N_CORES == 0
    n_loc = n_out // N_CORES

    src = np.asarray(edge_index[0], np.int64)
    dst = np.asarray(edge_index[1], np.int64)
    pos = np.full(N, -1, np.int64)
    pos[ids] = np.arange(n_out)
    dloc = pos[dst]
    sel = np.flatnonzero(dloc >= 0)
    dloc_sel = dloc[sel]
    core_of = dloc_sel // n_loc
    ordr = np.argsort(core_of, kind="stable")
    sel_sorted = sel[ordr]
    dloc_sorted = dloc_sel[ordr]
    bounds = np.searchsorted(core_of[ordr], np.arange(N_CORES + 1))
    counts = np.diff(bounds)
    e_cap = int(np.ceil(counts.max() / 128.0) * 128) + 128
    nblk = e_cap // 128
    sl0 = e_cap - 128  # self-loop block start

    # edge_attr mean slices, transposed to [128, 16, t_pp] (t contiguous)
    rows_pp = int(np.ceil(E / (N_CORES * 128.0)) * 128)
    t_pp = rows_pp // 128
    f_pp = t_pp * 16
    if N_CORES * rows_pp == E:
        ea_pad = edge_attr
    else:
        ea_pad = np.zeros((N_CORES * rows_pp, 16), np.float32)
        ea_pad[:E] = edge_attr
    nch = next((c for c in (4, 2, 1) if t_pp % c == 0))

    x_r = _round10(x)
    usrc = _round10((W.reshape(f_in, HEADS, C) * att_src[None]).sum(-1))
    udst = _round10((W.reshape(f_in, HEADS, C) * att_dst[None]).sum(-1))
    usd = _round10((W.reshape(f_in, HEADS, C)
                    * (att_src + att_dst)[None]).sum(-1))
    v16 = _round10((W_edge.reshape(-1, HEADS, C) * att_edge[None]).sum(-1))

    shared = {
        "w": _round10(W),
        "usrc": usrc, "udst": udst, "usd": usd, "v16": v16,
        "v16f": v16,
        "fcw": _round10(fc_W),
        "fcb": _round10(fc_b[None, :]),
        "onesr": np.ones((1, 128), np.float32),
        "onesc": np.ones((128, 1), np.float32),
    }

    in_maps = []
    for k in range(N_CORES):
        lo, hi = bounds[k], bounds[k + 1]
        nreal = hi - lo
        e_idx = sel_sorted[lo:hi]
        # layout: [real edges | pad | self-loop block: n_loc loops + pad]
        srcs = np.empty(e_cap, np.int64)
        srcs[:nreal] = src[e_idx]
        srcs[nreal:sl0] = ids[k * n_loc]  # pad; zeroed below
        srcs[sl0:sl0 + n_loc] = ids[k * n_loc:(k + 1) * n_loc]
        srcs[sl0 + n_loc:] = ids[k * n_loc]  # pad; zeroed below
        dstl = np.full(e_cap, n_loc, np.int64)  # pad -> no onehot match
        dstl[:nreal] = dloc_sorted[lo:hi] - k * n_loc
        xe = x_r[srcs]
        xe[nreal:sl0] = 0.0
        xe[sl0 + n_loc:] = 0.0
        eat = np.zeros((e_cap, 16), np.float32)
        eat[:nreal] = edge_attr[e_idx]
        # onehot only for real-edge blocks (self-loop block is handled as a
        # diagonal update in stage D)
        onehot = (dstl[:sl0, None] == np.arange(n_loc)[None, :]) \
            .astype(np.float32)                       # [sl0, n_loc]
        mea = ea_pad[k * rows_pp:(k + 1) * rows_pp] \
            .reshape(128, t_pp, 16).transpose(0, 2, 1)
        m = {
            "xt": np.ascontiguousarray(xe.T),
            "eat": _round10(np.ascontiguousarray(eat.T)),
            "eqt": np.ascontiguousarray(onehot.T),
            "eqb": np.ascontiguousarray(
                onehot.reshape(nblk - 1, 128, n_loc).transpose(1, 0, 2)
                .reshape(128, (nblk - 1) * n_loc)),
            "mea": np.ascontiguousarray(mea.reshape(128, f_pp))
            .astype(ml_dtypes.bfloat16),
            "biasb": np.ascontiguousarray(
                np.broadcast_to(bias_gat, (n_loc, HC))),
        }
        m.update(shared)
        in_maps.append(m)

    meta = dict(n_loc=n_loc, e_cap=e_cap, f_pp=f_pp, nch=nch,
                inv_e=1.0 / float(E), n_out=n_out)
    return in_maps, meta


def kernel(**inputs):
    trace = bool(inputs.pop("_trace", False))
    from concourse.bass_utils import run_bass_kernel_spmd

    in_maps, meta = _host_prep(
        inputs["x"], inputs["edge_index"], inputs["edge_attr"],
        inputs["num_groups"], inputs["agents_per_group"],
        inputs["W"], inputs["att_src"], inputs["att_dst"],
        inputs["W_edge"], inputs["att_edge"], inputs["bias_gat"],
        inputs["fc_W"], inputs["fc_b"])

    key = (meta["n_loc"], meta["e_cap"], meta["f_pp"], meta["nch"],
           meta["inv_e"])
    nc = _CACHE.get(key)
    if nc is None:
        nc = _build_program(meta["n_loc"], meta["e_cap"], meta["f_pp"],
                            meta["nch"], meta["inv_e"])
        _CACHE[key] = nc

    res = run_bass_kernel_spmd(nc, in_maps, list(range(N_CORES)), trace=trace)
    kernel.last_result = res
    out = np.concatenate([res.results[k]["out"] for k in range(N_CORES)],
                         axis=0)
    return np.ascontiguousarray(out, dtype=np.float32)
